# revision 8
# baseline (speedup 1.0000x reference)
"""Multi-head causal attention (B=4, S=2048, D=1024, H=16, dk=dv=64) on 8
Trainium2 NeuronCores.

Sharding: batch (4) x head-group (2) -> 8 cores. Each core computes, for its
batch b and its 8 heads, the partial output (concat_g @ WO_g)^T in [D, S]
layout. Host sums the two head-group partials per batch, transposes, adds bO.

v3 schedule: projection / out-projection matmul "units" are interleaved as
fillers inside the attention kc loop at an adaptive rate so the PE never
idles. fv matmuls run one kc behind exp (software pipeline). Inputs arrive
in a handful of large DMAs ordered so the warm-up (c-outer, 4-bank)
projections start as early as possible. Denominator path per q-block: DMA
gather of the 8 denom rows, one reciprocal_approx_fast, bf16 cast, then one
selector matmul per head pair broadcasts the per-head reciprocals across
partitions in PSUM for a single fused normalize multiply.
"""

import sys

sys.path.insert(0, "/opt/trn_rl_repo")

import numpy as np
import ml_dtypes

import concourse.bass as bass
from concourse import bacc
import concourse.tile as tile
from concourse import mybir
from concourse.bass_utils import run_bass_kernel_spmd

F32 = mybir.dt.float32
BF16 = mybir.dt.bfloat16
EXP = mybir.ActivationFunctionType.Exp
CPY = mybir.ActivationFunctionType.Copy
ADT = BF16   # dtype for Q/K/V/attn tiles (attention matmuls)

S = 2048          # sequence length
D = 1024          # model dim
HG = 8            # heads per core
DK = 64           # head dim
JG = HG * DK      # 512 = projected dim per core
CK = D // 128     # 8 contraction chunks for projections
NJT = JG // 128   # 4 j-tiles (head pairs)
NSB = S // 512    # 4 s-blocks of 512
NST = S // 128    # 16 s-tiles of 128
NQB = S // 512    # 4 q-blocks of 512

FILL_RATE = {0: 6, 1: 4, 2: 3, 3: 1}

_NC_CACHE = {}


def build_nc(salt=""):
    nc = bacc.Bacc("TRN2", target_bir_lowering=False)

    XT_d = nc.declare_dram_parameter("XT", [D, S], BF16, isOutput=False)
    WQ_d = nc.declare_dram_parameter("WQ", [D, JG], BF16, isOutput=False)
    WK_d = nc.declare_dram_parameter("WK", [D, JG], BF16, isOutput=False)
    WV_d = nc.declare_dram_parameter("WV", [D, JG], BF16, isOutput=False)
    WO_d = nc.declare_dram_parameter("WO", [JG, D], BF16, isOutput=False)
    OUT_d = nc.declare_dram_parameter("OUTT", [D, S], F32, isOutput=True)

    # tril-inclusive mask in (k, q) orientation: mask[kr, qr] = 1 iff qr >= kr
    tri_np = (np.arange(128)[None, :] >= np.arange(128)[:, None]).astype(
        mybir.dt.np(ADT))
    TRI_d = nc.inline_tensor(tri_np, name="trimask" + salt)

    # selector for denominator broadcast: sel[r, hp, j] = 1 iff r is the
    # denominator row of the head owning partition j within head pair hp
    sel_np = np.zeros((HG, NJT, 128), dtype=mybir.dt.np(ADT))
    for hp in range(NJT):
        sel_np[2 * hp, hp, 0:64] = 1
        sel_np[2 * hp + 1, hp, 64:128] = 1
    SEL_d = nc.inline_tensor(sel_np, name="dsel" + salt)

    with tile.TileContext(nc) as tc:
        with tc.tile_pool(name="persist", bufs=1) as pp:
            tri = pp.tile([128, 128], ADT, tag="tri", name="tri")
            nc.sync.dma_start(tri[:], TRI_d[:])
            sel = pp.tile([HG, NJT, 128], ADT, tag="sel", name="sel")
            nc.sync.dma_start(sel[:], SEL_d[:])

            QT = [pp.tile([128, S], ADT, tag=f"qt{j}", name=f"qt{j}")
                  for j in range(NJT)]
            KT = [pp.tile([128, S], ADT, tag=f"kt{j}", name=f"kt{j}")
                  for j in range(NJT)]
            VB = [pp.tile([128, HG, DK + 1], ADT, tag=f"vb{s}", name=f"vb{s}")
                  for s in range(NST)]

            XT = pp.tile([128, CK, S], BF16, tag="xt", name="xt")
            WKt = pp.tile([128, CK, JG], BF16, tag="wk", name="wk")
            WVt = pp.tile([128, CK, JG], BF16, tag="wv", name="wv")
            WQt = pp.tile([128, CK, JG], BF16, tag="wq", name="wq")
            WOt = pp.tile([128, NJT, D], BF16, tag="wo", name="wo")

            # Input DMAs, few and ordered: the warm-up needs WK + X(:, 0:512)
            # first. Chunk-halves keep the first matmul gate small.
            def ld_w(dst, src_d, lo, hi):
                nc.sync.dma_start(
                    dst[:, lo:hi, :],
                    src_d[128 * lo : 128 * hi, :].rearrange(
                        "(c p) j -> p c j", p=128),
                )

            def ld_x(clo, chi, slo, shi):
                nc.sync.dma_start(
                    XT[:, clo:chi, slo:shi],
                    XT_d[128 * clo : 128 * chi, slo:shi].rearrange(
                        "(c p) s -> p c s", p=128),
                )

            ld_w(WKt, WK_d, 0, 4)
            ld_x(0, 4, 0, 512)
            ld_w(WKt, WK_d, 4, 8)
            ld_x(4, 8, 0, 512)
            ld_w(WVt, WV_d, 0, 8)
            ld_w(WQt, WQ_d, 0, 8)
            ld_x(0, 8, 512, 2048)
            ld_w(WOt, WO_d, 0, 4)

            onestg = pp.tile([128, HG], F32, tag="onestg", name="onestg")
            nc.gpsimd.memset(onestg[:], 1.0)

            CT = [
                [pp.tile([128, 512], ADT, tag=f"ct{qb}_{hp}",
                         name=f"ct{qb}_{hp}") for hp in range(NJT)]
                for qb in range(NQB)
            ]
            coll = [pp.tile([HG, 512], F32, tag=f"coll{qb}",
                            name=f"coll{qb}") for qb in range(NQB)]

            def vb_evac(st, src):
                nc.vector.tensor_copy(
                    VB[st][:, :, 0:DK],
                    src.rearrange("p (h d) -> p h d", h=HG),
                )
                nc.vector.tensor_copy(
                    VB[st][:, :, DK : DK + 1],
                    onestg[:].rearrange("p (h u) -> p h u", u=1),
                )

            # ---------- warm-up: c-outer projection batches ----------
            with tc.tile_pool(name="warm", bufs=1, space="PSUM") as wp:
                for phase in range(3):
                    wt = [wp.tile([128, 512], F32, tag=f"w{i}", name=f"w{i}",
                                  bufs=2) for i in range(4)]
                    for c in range(CK):
                        for i in range(4):
                            if phase == 0:    # KT j-tiles, s-block 0
                                lhs = WKt[:, c, 128 * i : 128 * (i + 1)]
                                rhs = XT[:, c, 0:512]
                            elif phase == 1:  # VB s-tiles 0..3
                                lhs = XT[:, c, 128 * i : 128 * (i + 1)]
                                rhs = WVt[:, c, :]
                            else:             # QT j-tiles, q-block 0
                                lhs = WQt[:, c, 128 * i : 128 * (i + 1)]
                                rhs = XT[:, c, 0:512]
                            nc.tensor.matmul(wt[i][:], lhs, rhs,
                                             start=(c == 0), stop=(c == CK - 1))
                    for i in range(4):
                        if phase == 0:
                            nc.vector.tensor_copy(KT[i][:, 0:512], wt[i][:])
                        elif phase == 1:
                            vb_evac(i, wt[i][:])
                        else:
                            nc.vector.tensor_copy(QT[i][:, 0:512], wt[i][:])

            with (
                tc.tile_pool(name="acc", bufs=1, space="PSUM") as pa,
                tc.tile_pool(name="attn_ps", bufs=1, space="PSUM") as ps,
            ):
                def fv_tile(p):
                    return ps.tile([128, 512], F32, tag=f"fv{p}",
                                   name=f"fv{p}")

                # ---------- filler units ----------
                # Each unit generator yields once per PE matmul; evacuations
                # ride with the final yield. Units are queued in consumption
                # order and drained a few matmuls per attention iteration.
                def proj_unit(W, dest, jt, sb):
                    pq = pa.tile([128, 512], F32, tag="acc", name="acc",
                                 bufs=2)
                    for c in range(CK):
                        nc.tensor.matmul(
                            pq[:],
                            W[:, c, 128 * jt : 128 * (jt + 1)],
                            XT[:, c, 512 * sb : 512 * (sb + 1)],
                            start=(c == 0),
                            stop=(c == CK - 1),
                        )
                        if c < CK - 1:
                            yield
                    nc.vector.tensor_copy(
                        dest[jt][:, 512 * sb : 512 * (sb + 1)], pq[:]
                    )
                    yield

                def vb_unit(st):
                    pv = pa.tile([128, 512], F32, tag="acc", name="acc",
                                 bufs=2)
                    for c in range(CK):
                        nc.tensor.matmul(
                            pv[:],
                            XT[:, c, 128 * st : 128 * (st + 1)],
                            WVt[:, c, :],
                            start=(c == 0),
                            stop=(c == CK - 1),
                        )
                        if c < CK - 1:
                            yield
                    vb_evac(st, pv[:])
                    yield

                def outproj_unit(qb, mt, evac_act):
                    po = pa.tile([128, 512], F32, tag="acc", name="acc",
                                 bufs=2)
                    for c in range(NJT):
                        nc.tensor.matmul(
                            po[:],
                            WOt[:, c, 128 * mt : 128 * (mt + 1)],
                            CT[qb][c][:],
                            start=(c == 0),
                            stop=(c == NJT - 1),
                        )
                        if c < NJT - 1:
                            yield
                    og = pp.tile([128, 512], F32, tag="ostg", name="ostg",
                                 bufs=4)
                    if evac_act:
                        nc.scalar.activation(og[:], po[:], CPY)
                    else:
                        nc.vector.tensor_copy(og[:], po[:])
                    nc.sync.dma_start(
                        OUT_d[128 * mt : 128 * (mt + 1),
                              512 * qb : 512 * (qb + 1)],
                        og[:],
                    )
                    yield

                fillers = []  # list of (label, generator)

                def queue_proj_for(qb):
                    if qb >= NQB:
                        return
                    lab = ("proj", qb)
                    for jt in range(NJT):
                        fillers.append((lab, proj_unit(WKt, KT, jt, qb)))
                    for i in range(NJT):
                        fillers.append((lab, vb_unit(4 * qb + i)))
                    for jt in range(NJT):
                        fillers.append((lab, proj_unit(WQt, QT, jt, qb)))

                def fill(n):
                    k = 0
                    while fillers and k < n:
                        try:
                            next(fillers[0][1])
                            k += 1
                        except StopIteration:
                            fillers.pop(0)

                def drain_label(lab):
                    while any(l == lab for l, _ in fillers):
                        g = fillers.pop(0)[1]
                        for _ in g:
                            pass

                def drain_all():
                    while fillers:
                        g = fillers.pop(0)[1]
                        for _ in g:
                            pass

                queue_proj_for(1)

                # ---------- attention main loop ----------
                for qb in range(NQB):
                    if qb > 0:
                        drain_label(("proj", qb))
                    rate = FILL_RATE[qb]
                    for hp in range(NJT):
                        nkc = 4 * qb + 4
                        fv = [fv_tile(p) for p in (0, 1)]
                        pend = None  # deferred fv inputs (kc, o, at-tile)
                        for kc in range(nkc):
                            o = max(0, 128 * kc - 512 * qb)
                            sc = ps.tile([128, 2, 512], F32, tag="sc",
                                         name="sc", bufs=2)
                            for p in (0, 1):
                                pr = 64 * p
                                nc.tensor.matmul(
                                    sc[:, p, o:512],
                                    KT[hp][pr : pr + 64,
                                           128 * kc : 128 * (kc + 1)],
                                    QT[hp][pr : pr + 64,
                                           512 * qb + o : 512 * (qb + 1)],
                                    start=True,
                                    stop=True,
                                )
                            at = pp.tile([128, 2, 512], ADT, tag="attnT",
                                         name="attnT", bufs=6)
                            nc.scalar.activation(at[:, :, o:512],
                                                 sc[:, :, o:512], EXP,
                                                 scale=0.125)
                            if kc >= 4 * qb:  # diagonal-crossing tile
                                for p in (0, 1):
                                    blk = at[:, p, o : o + 128]
                                    nc.vector.tensor_mul(blk, blk, tri[:])
                            fill((rate + 1) // 2)
                            if pend is not None:
                                pkc, po_, pat = pend
                                for p in (0, 1):
                                    nc.tensor.matmul(
                                        fv[p][0 : DK + 1, po_:512],
                                        VB[pkc][:, 2 * hp + p, :],
                                        pat[:, p, po_:512],
                                        start=(pkc == 0),
                                        stop=False,
                                    )
                            pend = (kc, o, at)
                            fill(rate - (rate + 1) // 2)
                        pkc, po_, pat = pend
                        for p in (0, 1):
                            nc.tensor.matmul(
                                fv[p][0 : DK + 1, po_:512],
                                VB[pkc][:, 2 * hp + p, :],
                                pat[:, p, po_:512],
                                start=(pkc == 0),
                                stop=True,
                            )
                        for p in (0, 1):
                            nc.vector.tensor_copy(
                                CT[qb][hp][64 * p : 64 * (p + 1), :],
                                fv[p][0:64, :],
                            )
                            dr = pp.tile([1, 512], F32, tag="dr", name="dr",
                                         bufs=4)
                            nc.vector.tensor_copy(dr[:], fv[p][64:65, :])
                            nc.sync.dma_start(
                                coll[qb][2 * hp + p : 2 * hp + p + 1, :], dr[:]
                            )

                    # ---- q-block tail: normalize CT by softmax denom ----
                    crc = pp.tile([HG, 512], F32, tag="crc", name="crc",
                                  bufs=2)
                    nc.vector.reciprocal_approx_fast(crc[:], coll[qb][:])
                    crb = pp.tile([HG, 512], ADT, tag="crb", name="crb",
                                  bufs=2)
                    nc.vector.tensor_copy(crb[:], crc[:])
                    for hp in range(NJT):
                        rb = fv_tile(hp % 2)
                        nc.tensor.matmul(rb[:], sel[:, hp, :], crb[:],
                                         start=True, stop=True)
                        nc.vector.tensor_mul(CT[qb][hp][:], CT[qb][hp][:],
                                             rb[:])

                    queue_proj_for(qb + 1)
                    lab = ("out", qb)
                    for mt in range(D // 128):
                        fillers.append(
                            (lab, outproj_unit(qb, mt, qb == NQB - 1)))

                drain_all()
    nc.finalize()
    return nc


def _get_nc():
    if "nc" not in _NC_CACHE:
        _NC_CACHE["nc"] = build_nc()
    return _NC_CACHE["nc"]


def _make_in_maps(XKV, WQ, WK, WV, WO):
    bf = ml_dtypes.bfloat16
    in_maps = []
    for core in range(8):
        b, g = core // 2, core % 2
        sl = slice(512 * g, 512 * (g + 1))
        in_maps.append(
            {
                "XT": np.ascontiguousarray(XKV[b].T).astype(bf),
                "WQ": np.ascontiguousarray(WQ[:, sl]).astype(bf),
                "WK": np.ascontiguousarray(WK[:, sl]).astype(bf),
                "WV": np.ascontiguousarray(WV[:, sl]).astype(bf),
                "WO": np.ascontiguousarray(WO[sl, :]).astype(bf),
            }
        )
    return in_maps


def _combine(results, bO):
    out = np.empty((4, S, D), dtype=np.float32)
    for b in range(4):
        acc = results[2 * b]["OUTT"] + results[2 * b + 1]["OUTT"]
        out[b] = acc.T + bO[None, :]
    return out


def kernel(XKV, WQ, WK, WV, WO, bO):
    XKV = np.asarray(XKV, dtype=np.float32)
    nc = _get_nc()
    in_maps = _make_in_maps(XKV, np.asarray(WQ), np.asarray(WK), np.asarray(WV),
                            np.asarray(WO))
    res = run_bass_kernel_spmd(nc, in_maps, list(range(8)))
    return _combine(res.results, np.asarray(bO, dtype=np.float32))


# revision 10
# speedup vs baseline: 1.2698x; 1.2698x over previous
"""Multi-head causal attention (B=4, S=2048, D=1024, H=16, dk=dv=64) on 8
Trainium2 NeuronCores.

Sharding: batch (4) x head-group (2) -> 8 cores. Each core computes, for its
batch b and its 8 heads, the partial output (concat_g @ WO_g)^T in [D, S]
layout. Host sums the two head-group partials per batch, transposes, adds bO.

v3 schedule: projection / out-projection matmul "units" are interleaved as
fillers inside the attention kc loop at an adaptive rate so the PE never
idles. fv matmuls run one kc behind exp (software pipeline). Inputs arrive
in a handful of large DMAs ordered so the warm-up (c-outer, 4-bank)
projections start as early as possible. Denominator path per q-block: DMA
gather of the 8 denom rows, one reciprocal_approx_fast, bf16 cast, then one
selector matmul per head pair broadcasts the per-head reciprocals across
partitions in PSUM for a single fused normalize multiply.
"""

import sys

sys.path.insert(0, "/opt/trn_rl_repo")

import numpy as np
import ml_dtypes

import concourse.bass as bass
from concourse import bacc
import concourse.tile as tile
from concourse import mybir
from concourse.bass_utils import run_bass_kernel_spmd

F32 = mybir.dt.float32
BF16 = mybir.dt.bfloat16
EXP = mybir.ActivationFunctionType.Exp
CPY = mybir.ActivationFunctionType.Copy
ADT = BF16   # dtype for Q/K/V/attn tiles (attention matmuls)

S = 2048          # sequence length
D = 1024          # model dim
HG = 8            # heads per core
DK = 64           # head dim
JG = HG * DK      # 512 = projected dim per core
CK = D // 128     # 8 contraction chunks for projections
NJT = JG // 128   # 4 j-tiles (head pairs)
NSB = S // 512    # 4 s-blocks of 512
NST = S // 128    # 16 s-tiles of 128
NQB = S // 512    # 4 q-blocks of 512

FILL_RATE = {0: 6, 1: 4, 2: 3, 3: 1}

_NC_CACHE = {}


def build_nc(salt=""):
    nc = bacc.Bacc("TRN2", target_bir_lowering=False)

    XT_d = nc.declare_dram_parameter("XT", [D, S], BF16, isOutput=False)
    WQ_d = nc.declare_dram_parameter("WQ", [D, JG], BF16, isOutput=False)
    WK_d = nc.declare_dram_parameter("WK", [D, JG], BF16, isOutput=False)
    WV_d = nc.declare_dram_parameter("WV", [D, JG], BF16, isOutput=False)
    WO_d = nc.declare_dram_parameter("WO", [JG, D], BF16, isOutput=False)
    OUT_d = nc.declare_dram_parameter("OUTT", [D, S], F32, isOutput=True)

    # tril-inclusive mask in (k, q) orientation: mask[kr, qr] = 1 iff qr >= kr
    tri_np = (np.arange(128)[None, :] >= np.arange(128)[:, None]).astype(
        mybir.dt.np(ADT))
    TRI_d = nc.inline_tensor(tri_np, name="trimask" + salt)

    # selector for denominator broadcast: sel[r, hp, j] = 1 iff r is the
    # denominator row of the head owning partition j within head pair hp
    sel_np = np.zeros((HG, NJT, 128), dtype=mybir.dt.np(ADT))
    for hp in range(NJT):
        sel_np[2 * hp, hp, 0:64] = 1
        sel_np[2 * hp + 1, hp, 64:128] = 1
    SEL_d = nc.inline_tensor(sel_np, name="dsel" + salt)

    with tile.TileContext(nc) as tc:
        with tc.tile_pool(name="persist", bufs=1) as pp:
            tri = pp.tile([128, 128], ADT, tag="tri", name="tri")
            nc.sync.dma_start(tri[:], TRI_d[:])
            sel = pp.tile([HG, NJT, 128], ADT, tag="sel", name="sel")
            nc.sync.dma_start(sel[:], SEL_d[:])

            QT = [pp.tile([128, S], ADT, tag=f"qt{j}", name=f"qt{j}")
                  for j in range(NJT)]
            KT = [pp.tile([128, S], ADT, tag=f"kt{j}", name=f"kt{j}")
                  for j in range(NJT)]
            VB = [pp.tile([128, HG, DK + 1], ADT, tag=f"vb{s}", name=f"vb{s}")
                  for s in range(NST)]

            XT = pp.tile([128, CK, S], BF16, tag="xt", name="xt")
            WKt = pp.tile([128, CK, JG], BF16, tag="wk", name="wk")
            WVt = pp.tile([128, CK, JG], BF16, tag="wv", name="wv")
            WQt = pp.tile([128, CK, JG], BF16, tag="wq", name="wq")
            WOt = pp.tile([128, NJT, D], BF16, tag="wo", name="wo")

            # Input DMAs, few and ordered: the warm-up needs WK + X(:, 0:512)
            # first. Chunk-halves keep the first matmul gate small.
            def ld_w(dst, src_d, lo, hi):
                nc.sync.dma_start(
                    dst[:, lo:hi, :],
                    src_d[128 * lo : 128 * hi, :].rearrange(
                        "(c p) j -> p c j", p=128),
                )

            def ld_x(clo, chi, slo, shi):
                nc.sync.dma_start(
                    XT[:, clo:chi, slo:shi],
                    XT_d[128 * clo : 128 * chi, slo:shi].rearrange(
                        "(c p) s -> p c s", p=128),
                )

            for c in range(CK):
                ld_w(WKt, WK_d, c, c + 1)
                ld_x(c, c + 1, 0, 512)
                ld_w(WVt, WV_d, c, c + 1)
                ld_w(WQt, WQ_d, c, c + 1)
            for c in range(CK):
                ld_x(c, c + 1, 512, 2048)
            ld_w(WOt, WO_d, 0, 4)

            onestg = pp.tile([128, HG], F32, tag="onestg", name="onestg")
            nc.gpsimd.memset(onestg[:], 1.0)

            CT = [
                [pp.tile([128, 512], ADT, tag=f"ct{qb}_{hp}",
                         name=f"ct{qb}_{hp}") for hp in range(NJT)]
                for qb in range(NQB)
            ]
            coll = [pp.tile([HG, 512], F32, tag=f"coll{qb}",
                            name=f"coll{qb}") for qb in range(NQB)]

            def vb_evac(st, src):
                nc.vector.tensor_copy(
                    VB[st][:, :, 0:DK],
                    src.rearrange("p (h d) -> p h d", h=HG),
                )
                nc.vector.tensor_copy(
                    VB[st][:, :, DK : DK + 1],
                    onestg[:].rearrange("p (h u) -> p h u", u=1),
                )

            # ---------- warm-up: c-outer projection batches ----------
            with tc.tile_pool(name="warm", bufs=1, space="PSUM") as wp:
                for phase in range(3):
                    wt = [wp.tile([128, 512], F32, tag=f"w{i}", name=f"w{i}",
                                  bufs=2) for i in range(4)]
                    for c in range(CK):
                        for i in range(4):
                            if phase == 0:    # KT j-tiles, s-block 0
                                lhs = WKt[:, c, 128 * i : 128 * (i + 1)]
                                rhs = XT[:, c, 0:512]
                            elif phase == 1:  # VB s-tiles 0..3
                                lhs = XT[:, c, 128 * i : 128 * (i + 1)]
                                rhs = WVt[:, c, :]
                            else:             # QT j-tiles, q-block 0
                                lhs = WQt[:, c, 128 * i : 128 * (i + 1)]
                                rhs = XT[:, c, 0:512]
                            nc.tensor.matmul(wt[i][:], lhs, rhs,
                                             start=(c == 0), stop=(c == CK - 1))
                    for i in range(4):
                        if phase == 0:
                            nc.vector.tensor_copy(KT[i][:, 0:512], wt[i][:])
                        elif phase == 1:
                            vb_evac(i, wt[i][:])
                        else:
                            nc.vector.tensor_copy(QT[i][:, 0:512], wt[i][:])

            with (
                tc.tile_pool(name="acc", bufs=1, space="PSUM") as pa,
                tc.tile_pool(name="attn_ps", bufs=1, space="PSUM") as ps,
            ):
                def fv_tile(p):
                    return ps.tile([128, 512], F32, tag=f"fv{p}",
                                   name=f"fv{p}")

                # ---------- filler units ----------
                # Each unit generator yields once per PE matmul; evacuations
                # ride with the final yield. Units are queued in consumption
                # order and drained a few matmuls per attention iteration.
                def proj_unit(W, dest, jt, sb):
                    pq = pa.tile([128, 512], F32, tag="acc", name="acc",
                                 bufs=2)
                    for c in range(CK):
                        nc.tensor.matmul(
                            pq[:],
                            W[:, c, 128 * jt : 128 * (jt + 1)],
                            XT[:, c, 512 * sb : 512 * (sb + 1)],
                            start=(c == 0),
                            stop=(c == CK - 1),
                        )
                        if c < CK - 1:
                            yield
                    nc.vector.tensor_copy(
                        dest[jt][:, 512 * sb : 512 * (sb + 1)], pq[:]
                    )
                    yield

                def vb_unit(st):
                    pv = pa.tile([128, 512], F32, tag="acc", name="acc",
                                 bufs=2)
                    for c in range(CK):
                        nc.tensor.matmul(
                            pv[:],
                            XT[:, c, 128 * st : 128 * (st + 1)],
                            WVt[:, c, :],
                            start=(c == 0),
                            stop=(c == CK - 1),
                        )
                        if c < CK - 1:
                            yield
                    vb_evac(st, pv[:])
                    yield

                def outproj_unit(qb, mt, evac_act):
                    po = pa.tile([128, 512], F32, tag="acc", name="acc",
                                 bufs=2)
                    for c in range(NJT):
                        nc.tensor.matmul(
                            po[:],
                            WOt[:, c, 128 * mt : 128 * (mt + 1)],
                            CT[qb][c][:],
                            start=(c == 0),
                            stop=(c == NJT - 1),
                        )
                        if c < NJT - 1:
                            yield
                    og = pp.tile([128, 512], F32, tag="ostg", name="ostg",
                                 bufs=4)
                    if evac_act:
                        nc.scalar.activation(og[:], po[:], CPY)
                    else:
                        nc.vector.tensor_copy(og[:], po[:])
                    nc.sync.dma_start(
                        OUT_d[128 * mt : 128 * (mt + 1),
                              512 * qb : 512 * (qb + 1)],
                        og[:],
                    )
                    yield

                fillers = []  # list of (label, generator)

                def queue_proj_for(qb):
                    if qb >= NQB:
                        return
                    lab = ("proj", qb)
                    for jt in range(NJT):
                        fillers.append((lab, proj_unit(WKt, KT, jt, qb)))
                    for i in range(NJT):
                        fillers.append((lab, vb_unit(4 * qb + i)))
                    for jt in range(NJT):
                        fillers.append((lab, proj_unit(WQt, QT, jt, qb)))

                def fill(n):
                    k = 0
                    while fillers and k < n:
                        try:
                            next(fillers[0][1])
                            k += 1
                        except StopIteration:
                            fillers.pop(0)

                def drain_label(lab):
                    while any(l == lab for l, _ in fillers):
                        g = fillers.pop(0)[1]
                        for _ in g:
                            pass

                def drain_all():
                    while fillers:
                        g = fillers.pop(0)[1]
                        for _ in g:
                            pass

                queue_proj_for(1)

                # ---------- attention main loop ----------
                for qb in range(NQB):
                    if qb > 0:
                        drain_label(("proj", qb))
                    rate = FILL_RATE[qb]
                    for hp in range(NJT):
                        nkc = 4 * qb + 4
                        fv = [fv_tile(p) for p in (0, 1)]
                        pend = None  # deferred fv inputs (kc, o, at-tile)
                        for kc in range(nkc):
                            o = max(0, 128 * kc - 512 * qb)
                            sc = ps.tile([128, 2, 512], F32, tag="sc",
                                         name="sc", bufs=2)
                            for p in (0, 1):
                                pr = 64 * p
                                nc.tensor.matmul(
                                    sc[:, p, o:512],
                                    KT[hp][pr : pr + 64,
                                           128 * kc : 128 * (kc + 1)],
                                    QT[hp][pr : pr + 64,
                                           512 * qb + o : 512 * (qb + 1)],
                                    start=True,
                                    stop=True,
                                )
                            at = pp.tile([128, 2, 512], ADT, tag="attnT",
                                         name="attnT", bufs=6)
                            nc.scalar.activation(at[:, :, o:512],
                                                 sc[:, :, o:512], EXP,
                                                 scale=0.125)
                            if kc >= 4 * qb:  # diagonal-crossing tile
                                for p in (0, 1):
                                    blk = at[:, p, o : o + 128]
                                    nc.vector.tensor_mul(blk, blk, tri[:])
                            fill((rate + 1) // 2)
                            if pend is not None:
                                pkc, po_, pat = pend
                                for p in (0, 1):
                                    nc.tensor.matmul(
                                        fv[p][0 : DK + 1, po_:512],
                                        VB[pkc][:, 2 * hp + p, :],
                                        pat[:, p, po_:512],
                                        start=(pkc == 0),
                                        stop=False,
                                    )
                            pend = (kc, o, at)
                            fill(rate - (rate + 1) // 2)
                        pkc, po_, pat = pend
                        for p in (0, 1):
                            nc.tensor.matmul(
                                fv[p][0 : DK + 1, po_:512],
                                VB[pkc][:, 2 * hp + p, :],
                                pat[:, p, po_:512],
                                start=(pkc == 0),
                                stop=True,
                            )
                        for p in (0, 1):
                            nc.vector.tensor_copy(
                                CT[qb][hp][64 * p : 64 * (p + 1), :],
                                fv[p][0:64, :],
                            )
                            dr = pp.tile([1, 512], F32, tag="dr", name="dr",
                                         bufs=4)
                            nc.vector.tensor_copy(dr[:], fv[p][64:65, :])
                            nc.sync.dma_start(
                                coll[qb][2 * hp + p : 2 * hp + p + 1, :], dr[:]
                            )

                    # ---- q-block tail: normalize CT by softmax denom ----
                    crc = pp.tile([HG, 512], F32, tag="crc", name="crc",
                                  bufs=2)
                    nc.vector.reciprocal_approx_fast(crc[:], coll[qb][:])
                    crb = pp.tile([HG, 512], ADT, tag="crb", name="crb",
                                  bufs=2)
                    nc.vector.tensor_copy(crb[:], crc[:])
                    for hp in range(NJT):
                        rb = fv_tile(hp % 2)
                        nc.tensor.matmul(rb[:], sel[:, hp, :], crb[:],
                                         start=True, stop=True)
                        nc.vector.tensor_mul(CT[qb][hp][:], CT[qb][hp][:],
                                             rb[:])

                    queue_proj_for(qb + 2)
                    lab = ("out", qb)
                    for mt in range(D // 128):
                        fillers.append(
                            (lab, outproj_unit(qb, mt, qb == NQB - 1)))

                drain_all()
    nc.finalize()
    return nc


def _get_nc():
    if "nc" not in _NC_CACHE:
        _NC_CACHE["nc"] = build_nc()
    return _NC_CACHE["nc"]


def _make_in_maps(XKV, WQ, WK, WV, WO):
    bf = ml_dtypes.bfloat16
    in_maps = []
    for core in range(8):
        b, g = core // 2, core % 2
        sl = slice(512 * g, 512 * (g + 1))
        in_maps.append(
            {
                "XT": np.ascontiguousarray(XKV[b].T).astype(bf),
                "WQ": np.ascontiguousarray(WQ[:, sl]).astype(bf),
                "WK": np.ascontiguousarray(WK[:, sl]).astype(bf),
                "WV": np.ascontiguousarray(WV[:, sl]).astype(bf),
                "WO": np.ascontiguousarray(WO[sl, :]).astype(bf),
            }
        )
    return in_maps


def _combine(results, bO):
    out = np.empty((4, S, D), dtype=np.float32)
    for b in range(4):
        acc = results[2 * b]["OUTT"] + results[2 * b + 1]["OUTT"]
        out[b] = acc.T + bO[None, :]
    return out


def kernel(XKV, WQ, WK, WV, WO, bO):
    XKV = np.asarray(XKV, dtype=np.float32)
    nc = _get_nc()
    in_maps = _make_in_maps(XKV, np.asarray(WQ), np.asarray(WK), np.asarray(WV),
                            np.asarray(WO))
    res = run_bass_kernel_spmd(nc, in_maps, list(range(8)))
    return _combine(res.results, np.asarray(bO, dtype=np.float32))


# revision 14
# speedup vs baseline: 1.2835x; 1.0108x over previous
"""Multi-head causal attention (B=4, S=2048, D=1024, H=16, dk=dv=64) on 8
Trainium2 NeuronCores.

Sharding: batch (4) x head-group (2) -> 8 cores. Each core computes, for its
batch b and its 8 heads, the partial output (concat_g @ WO_g)^T in [D, S]
layout. Host sums the two head-group partials per batch, transposes, adds bO.

v3 schedule: projection / out-projection matmul "units" are interleaved as
fillers inside the attention kc loop at an adaptive rate so the PE never
idles. fv matmuls run one kc behind exp (software pipeline). Inputs arrive
in a handful of large DMAs ordered so the warm-up (c-outer, 4-bank)
projections start as early as possible. Denominator path per q-block: DMA
gather of the 8 denom rows, one reciprocal_approx_fast, bf16 cast, then one
selector matmul per head pair broadcasts the per-head reciprocals across
partitions in PSUM for a single fused normalize multiply.
"""

import sys

sys.path.insert(0, "/opt/trn_rl_repo")

import numpy as np
import ml_dtypes

import concourse.bass as bass
from concourse import bacc
import concourse.tile as tile
from concourse import mybir
from concourse.bass_utils import run_bass_kernel_spmd

F32 = mybir.dt.float32
BF16 = mybir.dt.bfloat16
EXP = mybir.ActivationFunctionType.Exp
CPY = mybir.ActivationFunctionType.Copy
ADT = BF16   # dtype for Q/K/V/attn tiles (attention matmuls)

S = 2048          # sequence length
D = 1024          # model dim
HG = 8            # heads per core
DK = 64           # head dim
JG = HG * DK      # 512 = projected dim per core
CK = D // 128     # 8 contraction chunks for projections
NJT = JG // 128   # 4 j-tiles (head pairs)
NSB = S // 512    # 4 s-blocks of 512
NST = S // 128    # 16 s-tiles of 128
NQB = S // 512    # 4 q-blocks of 512

FILL_RATE = {0: 6, 1: 4, 2: 3, 3: 1}

_NC_CACHE = {}


def build_nc(salt=""):
    nc = bacc.Bacc("TRN2", target_bir_lowering=False)

    XT_d = nc.declare_dram_parameter("XT", [D, S], BF16, isOutput=False)
    WQ_d = nc.declare_dram_parameter("WQ", [D, JG], BF16, isOutput=False)
    WK_d = nc.declare_dram_parameter("WK", [D, JG], BF16, isOutput=False)
    WV_d = nc.declare_dram_parameter("WV", [D, JG], BF16, isOutput=False)
    WO_d = nc.declare_dram_parameter("WO", [JG, D], BF16, isOutput=False)
    OUT_d = nc.declare_dram_parameter("OUTT", [D, S], F32, isOutput=True)

    # tril-inclusive mask in (k, q) orientation: mask[kr, qr] = 1 iff qr >= kr
    tri_np = (np.arange(128)[None, :] >= np.arange(128)[:, None]).astype(
        mybir.dt.np(ADT))
    TRI_d = nc.inline_tensor(tri_np, name="trimask" + salt)

    with tile.TileContext(nc) as tc:
        with tc.tile_pool(name="persist", bufs=1) as pp:
            tri = pp.tile([128, 128], ADT, tag="tri", name="tri")
            nc.sync.dma_start(tri[:], TRI_d[:])

            QT = [pp.tile([128, S], ADT, tag=f"qt{j}", name=f"qt{j}")
                  for j in range(NJT)]
            KT = [pp.tile([128, S], ADT, tag=f"kt{j}", name=f"kt{j}")
                  for j in range(NJT)]
            VB = [pp.tile([128, HG, DK + 1], ADT, tag=f"vb{s}", name=f"vb{s}")
                  for s in range(NST)]

            XT = pp.tile([128, CK, S], BF16, tag="xt", name="xt")
            WKt = pp.tile([128, CK, JG], BF16, tag="wk", name="wk")
            WVt = pp.tile([128, CK, JG], BF16, tag="wv", name="wv")
            WQt = pp.tile([128, CK, JG], BF16, tag="wq", name="wq")
            WOt = pp.tile([128, NJT, D], BF16, tag="wo", name="wo")

            # Input DMAs, few and ordered: the warm-up needs WK + X(:, 0:512)
            # first. Chunk-halves keep the first matmul gate small.
            def ld_w(dst, src_d, lo, hi):
                nc.sync.dma_start(
                    dst[:, lo:hi, :],
                    src_d[128 * lo : 128 * hi, :].rearrange(
                        "(c p) j -> p c j", p=128),
                )

            def ld_x(clo, chi, slo, shi):
                nc.sync.dma_start(
                    XT[:, clo:chi, slo:shi],
                    XT_d[128 * clo : 128 * chi, slo:shi].rearrange(
                        "(c p) s -> p c s", p=128),
                )

            for c in range(CK):
                ld_w(WKt, WK_d, c, c + 1)
                ld_x(c, c + 1, 0, 512)
                ld_w(WVt, WV_d, c, c + 1)
                ld_w(WQt, WQ_d, c, c + 1)
            for c in range(CK):
                ld_x(c, c + 1, 512, 2048)
            ld_w(WOt, WO_d, 0, 4)

            onestg = pp.tile([128, HG], F32, tag="onestg", name="onestg")
            nc.gpsimd.memset(onestg[:], 1.0)

            CT = [
                [pp.tile([128, 512], ADT, tag=f"ct{qb}_{hp}",
                         name=f"ct{qb}_{hp}") for hp in range(NJT)]
                for qb in range(NQB)
            ]

            def vb_evac(st, src):
                nc.vector.tensor_copy(
                    VB[st][:, :, 0:DK],
                    src.rearrange("p (h d) -> p h d", h=HG),
                )
                nc.vector.tensor_copy(
                    VB[st][:, :, DK : DK + 1],
                    onestg[:].rearrange("p (h u) -> p h u", u=1),
                )

            # ---------- warm-up: c-outer projection batches ----------
            with tc.tile_pool(name="warm", bufs=1, space="PSUM") as wp:
                for phase in range(3):
                    wt = [wp.tile([128, 512], F32, tag=f"w{i}", name=f"w{i}",
                                  bufs=2) for i in range(4)]
                    for c in range(CK):
                        for i in range(4):
                            if phase == 0:    # KT j-tiles, s-block 0
                                lhs = WKt[:, c, 128 * i : 128 * (i + 1)]
                                rhs = XT[:, c, 0:512]
                            elif phase == 1:  # VB s-tiles 0..3
                                lhs = XT[:, c, 128 * i : 128 * (i + 1)]
                                rhs = WVt[:, c, :]
                            else:             # QT j-tiles, q-block 0
                                lhs = WQt[:, c, 128 * i : 128 * (i + 1)]
                                rhs = XT[:, c, 0:512]
                            nc.tensor.matmul(wt[i][:], lhs, rhs,
                                             start=(c == 0), stop=(c == CK - 1))
                    for i in range(4):
                        if phase == 0:
                            nc.vector.tensor_copy(KT[i][:, 0:512], wt[i][:])
                        elif phase == 1:
                            vb_evac(i, wt[i][:])
                        else:
                            nc.vector.tensor_copy(QT[i][:, 0:512], wt[i][:])

            with (
                tc.tile_pool(name="acc", bufs=1, space="PSUM") as pa,
                tc.tile_pool(name="attn_ps", bufs=1, space="PSUM") as ps,
            ):
                def fv_tile(p):
                    return ps.tile([128, 512], F32, tag=f"fv{p}",
                                   name=f"fv{p}")

                # ---------- filler units ----------
                # Each unit generator yields once per PE matmul; evacuations
                # ride with the final yield. Units are queued in consumption
                # order and drained a few matmuls per attention iteration.
                def proj_unit(W, dest, jt, sb):
                    pq = pa.tile([128, 512], F32, tag="acc", name="acc",
                                 bufs=2)
                    for c in range(CK):
                        nc.tensor.matmul(
                            pq[:],
                            W[:, c, 128 * jt : 128 * (jt + 1)],
                            XT[:, c, 512 * sb : 512 * (sb + 1)],
                            start=(c == 0),
                            stop=(c == CK - 1),
                        )
                        if c < CK - 1:
                            yield
                    nc.vector.tensor_copy(
                        dest[jt][:, 512 * sb : 512 * (sb + 1)], pq[:]
                    )
                    yield

                def vb_unit(st):
                    pv = pa.tile([128, 512], F32, tag="acc", name="acc",
                                 bufs=2)
                    for c in range(CK):
                        nc.tensor.matmul(
                            pv[:],
                            XT[:, c, 128 * st : 128 * (st + 1)],
                            WVt[:, c, :],
                            start=(c == 0),
                            stop=(c == CK - 1),
                        )
                        if c < CK - 1:
                            yield
                    vb_evac(st, pv[:])
                    yield

                def outproj_unit(qb, mt, evac_act):
                    po = pa.tile([128, 512], F32, tag="acc", name="acc",
                                 bufs=2)
                    for c in range(NJT):
                        nc.tensor.matmul(
                            po[:],
                            WOt[:, c, 128 * mt : 128 * (mt + 1)],
                            CT[qb][c][:],
                            start=(c == 0),
                            stop=(c == NJT - 1),
                        )
                        if c < NJT - 1:
                            yield
                    og = pp.tile([128, 512], F32, tag="ostg", name="ostg",
                                 bufs=4)
                    if evac_act:
                        nc.scalar.activation(og[:], po[:], CPY)
                    else:
                        nc.vector.tensor_copy(og[:], po[:])
                    nc.sync.dma_start(
                        OUT_d[128 * mt : 128 * (mt + 1),
                              512 * qb : 512 * (qb + 1)],
                        og[:],
                    )
                    yield

                fillers = []  # list of (label, generator)

                def queue_proj_for(qb):
                    if qb >= NQB:
                        return
                    lab = ("proj", qb)
                    for jt in range(NJT):
                        fillers.append((lab, proj_unit(WKt, KT, jt, qb)))
                    for i in range(NJT):
                        fillers.append((lab, vb_unit(4 * qb + i)))
                    for jt in range(NJT):
                        fillers.append((lab, proj_unit(WQt, QT, jt, qb)))

                def fill(n):
                    k = 0
                    while fillers and k < n:
                        try:
                            next(fillers[0][1])
                            k += 1
                        except StopIteration:
                            fillers.pop(0)

                def drain_label(lab):
                    while any(l == lab for l, _ in fillers):
                        g = fillers.pop(0)[1]
                        for _ in g:
                            pass

                def drain_all():
                    while fillers:
                        g = fillers.pop(0)[1]
                        for _ in g:
                            pass

                queue_proj_for(1)

                # ---------- attention main loop ----------
                for qb in range(NQB):
                    if qb > 0:
                        drain_label(("proj", qb))
                    rate = FILL_RATE[qb]
                    for hp in range(NJT):
                        nkc = 4 * qb + 4
                        fv = [fv_tile(p) for p in (0, 1)]
                        pend = None  # deferred fv inputs (kc, o, at-tile)
                        for kc in range(nkc):
                            o = max(0, 128 * kc - 512 * qb)
                            sc = ps.tile([128, 2, 512], F32, tag="sc",
                                         name="sc", bufs=2)
                            for p in (0, 1):
                                pr = 64 * p
                                nc.tensor.matmul(
                                    sc[:, p, o:512],
                                    KT[hp][pr : pr + 64,
                                           128 * kc : 128 * (kc + 1)],
                                    QT[hp][pr : pr + 64,
                                           512 * qb + o : 512 * (qb + 1)],
                                    start=True,
                                    stop=True,
                                )
                            at = pp.tile([128, 2, 512], ADT, tag="attnT",
                                         name="attnT", bufs=6)
                            nc.scalar.activation(at[:, :, o:512],
                                                 sc[:, :, o:512], EXP,
                                                 scale=0.125)
                            if kc >= 4 * qb:  # diagonal-crossing tile
                                for p in (0, 1):
                                    blk = at[:, p, o : o + 128]
                                    nc.vector.tensor_mul(blk, blk, tri[:])
                            fill((rate + 1) // 2)
                            if pend is not None:
                                pkc, po_, pat = pend
                                for p in (0, 1):
                                    nc.tensor.matmul(
                                        fv[p][0 : DK + 1, po_:512],
                                        VB[pkc][:, 2 * hp + p, :],
                                        pat[:, p, po_:512],
                                        start=(pkc == 0),
                                        stop=False,
                                    )
                            pend = (kc, o, at)
                            fill(rate - (rate + 1) // 2)
                        pkc, po_, pat = pend
                        for p in (0, 1):
                            nc.tensor.matmul(
                                fv[p][0 : DK + 1, po_:512],
                                VB[pkc][:, 2 * hp + p, :],
                                pat[:, p, po_:512],
                                start=(pkc == 0),
                                stop=True,
                            )
                        # eager per-head-pair softmax normalization: the
                        # reciprocal reads the denominator row straight from
                        # PSUM, Pool broadcasts it across partitions, and one
                        # multiply per head rescales CT in place.
                        rbs = []
                        for p in (0, 1):
                            nc.vector.tensor_copy(
                                CT[qb][hp][64 * p : 64 * (p + 1), :],
                                fv[p][0:64, :],
                            )
                            dr = pp.tile([1, 512], F32, tag="dr", name="dr",
                                         bufs=4)
                            nc.vector.tensor_copy(dr[:], fv[p][64:65, :])
                            rcp = pp.tile([1, 512], F32, tag="rcp",
                                          name="rcp", bufs=4)
                            nc.vector.reciprocal_approx_fast(rcp[:], dr[:])
                            rb = pp.tile([128, 512], F32, tag=f"rb{p}",
                                         name=f"rb{p}", bufs=2)
                            nc.gpsimd.partition_broadcast(rb[:], rcp[:])
                            rbs.append(rb)
                        for p in (0, 1):
                            ct_sl = CT[qb][hp][64 * p : 64 * (p + 1), :]
                            nc.vector.tensor_mul(
                                ct_sl, ct_sl, rbs[p][64 * p : 64 * (p + 1), :]
                            )

                    queue_proj_for(qb + 2)
                    lab = ("out", qb)
                    for mt in range(D // 128):
                        fillers.append(
                            (lab, outproj_unit(qb, mt, qb == NQB - 1)))

                drain_all()
    nc.finalize()
    return nc


def _get_nc():
    if "nc" not in _NC_CACHE:
        _NC_CACHE["nc"] = build_nc()
    return _NC_CACHE["nc"]


def _make_in_maps(XKV, WQ, WK, WV, WO):
    bf = ml_dtypes.bfloat16
    in_maps = []
    for core in range(8):
        b, g = core // 2, core % 2
        sl = slice(512 * g, 512 * (g + 1))
        in_maps.append(
            {
                "XT": np.ascontiguousarray(XKV[b].T).astype(bf),
                "WQ": np.ascontiguousarray(WQ[:, sl]).astype(bf),
                "WK": np.ascontiguousarray(WK[:, sl]).astype(bf),
                "WV": np.ascontiguousarray(WV[:, sl]).astype(bf),
                "WO": np.ascontiguousarray(WO[sl, :]).astype(bf),
            }
        )
    return in_maps


def _combine(results, bO):
    out = np.empty((4, S, D), dtype=np.float32)
    for b in range(4):
        acc = results[2 * b]["OUTT"] + results[2 * b + 1]["OUTT"]
        out[b] = acc.T + bO[None, :]
    return out


def kernel(XKV, WQ, WK, WV, WO, bO):
    XKV = np.asarray(XKV, dtype=np.float32)
    nc = _get_nc()
    in_maps = _make_in_maps(XKV, np.asarray(WQ), np.asarray(WK), np.asarray(WV),
                            np.asarray(WO))
    res = run_bass_kernel_spmd(nc, in_maps, list(range(8)))
    return _combine(res.results, np.asarray(bO, dtype=np.float32))


# revision 18
# speedup vs baseline: 1.3102x; 1.0208x over previous
"""Multi-head causal attention (B=4, S=2048, D=1024, H=16, dk=dv=64) on 8
Trainium2 NeuronCores.

Sharding: batch (4) x head-group (2) -> 8 cores. Each core computes, for its
batch b and its 8 heads, the partial output (concat_g @ WO_g)^T in [D, S]
layout. Host sums the two head-group partials per batch, transposes, adds bO.

v3 schedule: projection / out-projection matmul "units" are interleaved as
fillers inside the attention kc loop at an adaptive rate so the PE never
idles. fv matmuls run one kc behind exp (software pipeline). Inputs arrive
in a handful of large DMAs ordered so the warm-up (c-outer, 4-bank)
projections start as early as possible. Denominator path per q-block: DMA
gather of the 8 denom rows, one reciprocal_approx_fast, bf16 cast, then one
selector matmul per head pair broadcasts the per-head reciprocals across
partitions in PSUM for a single fused normalize multiply.
"""

import sys

sys.path.insert(0, "/opt/trn_rl_repo")

import numpy as np
import ml_dtypes

import concourse.bass as bass
from concourse import bacc
import concourse.tile as tile
from concourse import mybir
from concourse.bass_utils import run_bass_kernel_spmd

F32 = mybir.dt.float32
BF16 = mybir.dt.bfloat16
EXP = mybir.ActivationFunctionType.Exp
CPY = mybir.ActivationFunctionType.Copy
ADT = BF16   # dtype for Q/K/V/attn tiles (attention matmuls)

S = 2048          # sequence length
D = 1024          # model dim
HG = 8            # heads per core
DK = 64           # head dim
JG = HG * DK      # 512 = projected dim per core
CK = D // 128     # 8 contraction chunks for projections
NJT = JG // 128   # 4 j-tiles (head pairs)
NSB = S // 512    # 4 s-blocks of 512
NST = S // 128    # 16 s-tiles of 128
NQB = S // 512    # 4 q-blocks of 512

FILL_RATE = {0: 6, 1: 4, 2: 3, 3: 1}

_NC_CACHE = {}


def build_nc(salt=""):
    nc = bacc.Bacc("TRN2", target_bir_lowering=False)

    XT_d = nc.declare_dram_parameter("XT", [D, S], BF16, isOutput=False)
    WQ_d = nc.declare_dram_parameter("WQ", [D, JG], BF16, isOutput=False)
    WK_d = nc.declare_dram_parameter("WK", [D, JG], BF16, isOutput=False)
    WV_d = nc.declare_dram_parameter("WV", [D, JG], BF16, isOutput=False)
    WO_d = nc.declare_dram_parameter("WO", [JG, D], BF16, isOutput=False)
    OUT_d = nc.declare_dram_parameter("OUTT", [D, S], F32, isOutput=True)

    # tril-inclusive mask in (k, q) orientation: mask[kr, qr] = 1 iff qr >= kr
    tri_np = (np.arange(128)[None, :] >= np.arange(128)[:, None]).astype(
        mybir.dt.np(ADT))
    TRI_d = nc.inline_tensor(tri_np, name="trimask" + salt)

    with tile.TileContext(nc) as tc:
        with tc.tile_pool(name="persist", bufs=1) as pp:
            tri = pp.tile([128, 128], ADT, tag="tri", name="tri")

            QT = [pp.tile([128, S], ADT, tag=f"qt{j}", name=f"qt{j}")
                  for j in range(NJT)]
            KT = [pp.tile([128, S], ADT, tag=f"kt{j}", name=f"kt{j}")
                  for j in range(NJT)]
            VB = [pp.tile([128, HG, DK + 1], ADT, tag=f"vb{s}", name=f"vb{s}")
                  for s in range(NST)]

            XT = pp.tile([128, CK, S], BF16, tag="xt", name="xt")
            WKt = pp.tile([128, CK, JG], BF16, tag="wk", name="wk")
            WVt = pp.tile([128, CK, JG], BF16, tag="wv", name="wv")
            WQt = pp.tile([128, CK, JG], BF16, tag="wq", name="wq")
            WOt = pp.tile([128, NJT, D], BF16, tag="wo", name="wo")

            # Input DMAs, few and ordered: the warm-up needs WK + X(:, 0:512)
            # first. Chunk-halves keep the first matmul gate small.
            def ld_w(dst, src_d, lo, hi):
                nc.sync.dma_start(
                    dst[:, lo:hi, :],
                    src_d[128 * lo : 128 * hi, :].rearrange(
                        "(c p) j -> p c j", p=128),
                )

            def ld_x(clo, chi, slo, shi):
                nc.sync.dma_start(
                    XT[:, clo:chi, slo:shi],
                    XT_d[128 * clo : 128 * chi, slo:shi].rearrange(
                        "(c p) s -> p c s", p=128),
                )

            for c in range(CK):
                ld_w(WKt, WK_d, c, c + 1)
                ld_x(c, c + 1, 0, 512)
                ld_w(WVt, WV_d, c, c + 1)
                ld_w(WQt, WQ_d, c, c + 1)
            nc.sync.dma_start(tri[:], TRI_d[:])
            for c in range(CK):
                ld_x(c, c + 1, 512, 2048)
            ld_w(WOt, WO_d, 0, 4)

            onestg = pp.tile([128, HG], F32, tag="onestg", name="onestg")
            nc.gpsimd.memset(onestg[:], 1.0)

            CT = [
                [pp.tile([128, 512], ADT, tag=f"ct{qb}_{hp}",
                         name=f"ct{qb}_{hp}") for hp in range(NJT)]
                for qb in range(NQB)
            ]

            def vb_evac(st, src):
                nc.vector.tensor_copy(
                    VB[st][:, :, 0:DK],
                    src.rearrange("p (h d) -> p h d", h=HG),
                )
                nc.vector.tensor_copy(
                    VB[st][:, :, DK : DK + 1],
                    onestg[:].rearrange("p (h u) -> p h u", u=1),
                )

            # ---------- warm-up: c-outer projection batches ----------
            with tc.tile_pool(name="warm", bufs=1, space="PSUM") as wp:
                for phase in range(3):
                    wt = [wp.tile([128, 512], F32, tag=f"w{i}", name=f"w{i}",
                                  bufs=2) for i in range(4)]
                    for c in range(CK):
                        for i in range(4):
                            if phase == 0:    # KT j-tiles, s-block 0
                                lhs = WKt[:, c, 128 * i : 128 * (i + 1)]
                                rhs = XT[:, c, 0:512]
                            elif phase == 1:  # VB s-tiles 0..3
                                lhs = XT[:, c, 128 * i : 128 * (i + 1)]
                                rhs = WVt[:, c, :]
                            else:             # QT j-tiles, q-block 0
                                lhs = WQt[:, c, 128 * i : 128 * (i + 1)]
                                rhs = XT[:, c, 0:512]
                            nc.tensor.matmul(wt[i][:], lhs, rhs,
                                             start=(c == 0), stop=(c == CK - 1))
                    for i in range(4):
                        if phase == 0:
                            nc.vector.tensor_copy(KT[i][:, 0:512], wt[i][:])
                        elif phase == 1:
                            vb_evac(i, wt[i][:])
                        else:
                            nc.vector.tensor_copy(QT[i][:, 0:512], wt[i][:])

            with (
                tc.tile_pool(name="acc", bufs=1, space="PSUM") as pa,
                tc.tile_pool(name="attn_ps", bufs=1, space="PSUM") as ps,
            ):
                def fv_tile(p):
                    return ps.tile([128, 512], F32, tag=f"fv{p}",
                                   name=f"fv{p}")

                # ---------- filler units ----------
                # Each unit generator yields once per PE matmul; evacuations
                # ride with the final yield. Units are queued in consumption
                # order and drained a few matmuls per attention iteration.
                def proj_unit(W, dest, jt, sb):
                    pq = pa.tile([128, 512], F32, tag="acc", name="acc",
                                 bufs=2)
                    for c in range(CK):
                        nc.tensor.matmul(
                            pq[:],
                            W[:, c, 128 * jt : 128 * (jt + 1)],
                            XT[:, c, 512 * sb : 512 * (sb + 1)],
                            start=(c == 0),
                            stop=(c == CK - 1),
                        )
                        if c < CK - 1:
                            yield
                    nc.vector.tensor_copy(
                        dest[jt][:, 512 * sb : 512 * (sb + 1)], pq[:]
                    )
                    yield

                def vb_unit(st):
                    pv = pa.tile([128, 512], F32, tag="acc", name="acc",
                                 bufs=2)
                    for c in range(CK):
                        nc.tensor.matmul(
                            pv[:],
                            XT[:, c, 128 * st : 128 * (st + 1)],
                            WVt[:, c, :],
                            start=(c == 0),
                            stop=(c == CK - 1),
                        )
                        if c < CK - 1:
                            yield
                    vb_evac(st, pv[:])
                    yield

                def outproj_unit(qb, mt, evac_act):
                    po = pa.tile([128, 512], F32, tag="acc", name="acc",
                                 bufs=2)
                    for i in range(NJT):
                        c = (i + mt) % NJT
                        nc.tensor.matmul(
                            po[:],
                            WOt[:, c, 128 * mt : 128 * (mt + 1)],
                            CT[qb][c][:],
                            start=(i == 0),
                            stop=(i == NJT - 1),
                        )
                        if i < NJT - 1:
                            yield
                    og = pp.tile([128, 512], F32, tag="ostg", name="ostg",
                                 bufs=4)
                    if evac_act:
                        nc.scalar.activation(og[:], po[:], CPY)
                    else:
                        nc.vector.tensor_copy(og[:], po[:])
                    nc.sync.dma_start(
                        OUT_d[128 * mt : 128 * (mt + 1),
                              512 * qb : 512 * (qb + 1)],
                        og[:],
                    )
                    yield

                fillers = []  # list of (label, generator)

                def queue_proj_for(qb):
                    if qb >= NQB:
                        return
                    lab = ("proj", qb)
                    for jt in range(NJT):
                        fillers.append((lab, proj_unit(WKt, KT, jt, qb)))
                    for i in range(NJT):
                        fillers.append((lab, vb_unit(4 * qb + i)))
                    for jt in range(NJT):
                        fillers.append((lab, proj_unit(WQt, QT, jt, qb)))

                def fill(n):
                    k = 0
                    while fillers and k < n:
                        try:
                            next(fillers[0][1])
                            k += 1
                        except StopIteration:
                            fillers.pop(0)

                def drain_label(lab):
                    while any(l == lab for l, _ in fillers):
                        g = fillers.pop(0)[1]
                        for _ in g:
                            pass

                def drain_all():
                    while fillers:
                        g = fillers.pop(0)[1]
                        for _ in g:
                            pass

                queue_proj_for(1)

                # ---------- attention main loop ----------
                for qb in range(NQB):
                    if qb > 0:
                        drain_label(("proj", qb))
                    rate = FILL_RATE[qb]
                    for hp in range(NJT):
                        nkc = 4 * qb + 4
                        fv = [fv_tile(p) for p in (0, 1)]
                        pend = None  # deferred fv inputs (kc, o, at-tile)
                        for kc in range(nkc):
                            o = max(0, 128 * kc - 512 * qb)
                            sc = ps.tile([128, 2, 512], F32, tag="sc",
                                         name="sc", bufs=2)
                            for p in (0, 1):
                                pr = 64 * p
                                nc.tensor.matmul(
                                    sc[:, p, o:512],
                                    KT[hp][pr : pr + 64,
                                           128 * kc : 128 * (kc + 1)],
                                    QT[hp][pr : pr + 64,
                                           512 * qb + o : 512 * (qb + 1)],
                                    start=True,
                                    stop=True,
                                )
                            at = pp.tile([128, 2, 512], ADT, tag="attnT",
                                         name="attnT", bufs=6)
                            nc.scalar.activation(at[:, :, o:512],
                                                 sc[:, :, o:512], EXP,
                                                 scale=0.125)
                            if kc >= 4 * qb:  # diagonal-crossing tile
                                for p in (0, 1):
                                    blk = at[:, p, o : o + 128]
                                    nc.vector.tensor_mul(blk, blk, tri[:])
                            fill((rate + 1) // 2)
                            if pend is not None:
                                pkc, po_, pat = pend
                                for p in (0, 1):
                                    nc.tensor.matmul(
                                        fv[p][0 : DK + 1, po_:512],
                                        VB[pkc][:, 2 * hp + p, :],
                                        pat[:, p, po_:512],
                                        start=(pkc == 0),
                                        stop=False,
                                    )
                            pend = (kc, o, at)
                            fill(rate - (rate + 1) // 2)
                        pkc, po_, pat = pend
                        for p in (0, 1):
                            nc.tensor.matmul(
                                fv[p][0 : DK + 1, po_:512],
                                VB[pkc][:, 2 * hp + p, :],
                                pat[:, p, po_:512],
                                start=(pkc == 0),
                                stop=True,
                            )
                        # eager per-head-pair softmax normalization: the
                        # reciprocal reads the denominator row straight from
                        # PSUM, Pool broadcasts it across partitions, and one
                        # multiply per head rescales CT in place.
                        rbs = []
                        for p in (0, 1):
                            nc.scalar.copy(
                                CT[qb][hp][64 * p : 64 * (p + 1), :],
                                fv[p][0:64, :],
                            )
                            dr = pp.tile([1, 512], F32, tag="dr", name="dr",
                                         bufs=4)
                            nc.vector.tensor_copy(dr[:], fv[p][64:65, :])
                            rcp = pp.tile([1, 512], F32, tag="rcp",
                                          name="rcp", bufs=4)
                            nc.vector.reciprocal_approx_fast(rcp[:], dr[:])
                            rb = pp.tile([128, 512], F32, tag=f"rb{p}",
                                         name=f"rb{p}", bufs=2)
                            nc.gpsimd.partition_broadcast(rb[:], rcp[:])
                            rbs.append(rb)
                        for p in (0, 1):
                            ct_sl = CT[qb][hp][64 * p : 64 * (p + 1), :]
                            nc.vector.tensor_mul(
                                ct_sl, ct_sl, rbs[p][64 * p : 64 * (p + 1), :]
                            )

                    queue_proj_for(qb + 2)
                    lab = ("out", qb)
                    for mt in range(D // 128):
                        fillers.append(
                            (lab, outproj_unit(qb, mt, qb == NQB - 1)))

                drain_all()
    nc.finalize()
    return nc


def _get_nc():
    if "nc" not in _NC_CACHE:
        _NC_CACHE["nc"] = build_nc()
    return _NC_CACHE["nc"]


def _make_in_maps(XKV, WQ, WK, WV, WO):
    bf = ml_dtypes.bfloat16
    in_maps = []
    for core in range(8):
        b, g = core // 2, core % 2
        sl = slice(512 * g, 512 * (g + 1))
        in_maps.append(
            {
                "XT": np.ascontiguousarray(XKV[b].T).astype(bf),
                "WQ": np.ascontiguousarray(WQ[:, sl]).astype(bf),
                "WK": np.ascontiguousarray(WK[:, sl]).astype(bf),
                "WV": np.ascontiguousarray(WV[:, sl]).astype(bf),
                "WO": np.ascontiguousarray(WO[sl, :]).astype(bf),
            }
        )
    return in_maps


def _combine(results, bO):
    out = np.empty((4, S, D), dtype=np.float32)
    for b in range(4):
        acc = results[2 * b]["OUTT"] + results[2 * b + 1]["OUTT"]
        out[b] = acc.T + bO[None, :]
    return out


def kernel(XKV, WQ, WK, WV, WO, bO):
    XKV = np.asarray(XKV, dtype=np.float32)
    nc = _get_nc()
    in_maps = _make_in_maps(XKV, np.asarray(WQ), np.asarray(WK), np.asarray(WV),
                            np.asarray(WO))
    res = run_bass_kernel_spmd(nc, in_maps, list(range(8)))
    return _combine(res.results, np.asarray(bO, dtype=np.float32))
